# revision 1
# baseline (speedup 1.0000x reference)
"""AdderNet BasicBlock kernel for Trainium2, co-sharded across 8 cores.

Per core (co-shard CO=8 of 64 output channels):
  conv[co,n,p] = -sum_{ci,kh,kw} |x[n,ci,p+k-1] - w[co,ci,kh,kw]|   (pad=1)
  BN train-mode over (n,h,w) per co, then ReLU.

Formulation: |d| = 2*relu(d) - d with d = x - w, so
  conv = -2*sum(relu(x-w)) + BoxX - S_w
    BoxX = sum_{ci,tap} x_patch  (PE ones-matmuls on x directly)
    S_w  = sum_{ci,tap} w[co]    (folded into PSUM-evacuation bias)

Engines:
  DVE: tensor_scalar(sub, max 0) -> relu(x-w), fp32 2x mode
  ACT: share of relu passes (activation Relu, bias=-w) + PSUM evacuation
  PE : ones-matmul reduce over ci partitions (128 rows = 2 image groups x 64 ci),
       PSUM accumulates 9 taps; float32r for full-rate streaming
  BN : conv bounced via DRAM to [(co,n), hw]; replicated-selector matmul stats.
"""
from contextlib import ExitStack

import numpy as np

import concourse.bass as bass
import concourse.tile as tile
import concourse.mybir as mybir

F32 = mybir.dt.float32
BF16 = mybir.dt.bfloat16
F32R = mybir.dt.float32r
BN_EPS = 1e-5

N, CI, H, W = 16, 64, 32, 32
CO = 8          # output channels per core
HW = H * W      # 1024
PADH, PADW = H + 2, W + 2  # 34


def split_multiwaits(nc, max_waits=1):
    """This container's walrus rejects >1 semaphore wait per instruction.
    Hoist extras into standalone NoOps on the same (in-order) engine."""
    n_split = 0
    for f in nc.m.functions:
        for b in f.blocks:
            insts = list(b.instructions)
            changed = False
            new = []
            for inst in insts:
                si = inst.sync_info
                waits = list(si.on_wait) if si and si.on_wait else []
                if len(waits) > max_waits:
                    changed = True
                    n_split += 1
                    for w in waits[: len(waits) - max_waits]:
                        new.append(mybir.InstNoOp(
                            name=nc.get_next_instruction_name(),
                            engine=inst.engine, ins=[], outs=[],
                            sync_info=mybir.SyncInfo(on_wait=[w], on_update=[]),
                        ))
                    inst.sync_info = mybir.SyncInfo(
                        on_wait=waits[len(waits) - max_waits:],
                        on_update=list(si.on_update) if si.on_update else [],
                    )
                new.append(inst)
            if changed:
                b.instructions = new
    return n_split


def build_nc(reduce_dtype="f32r", act_tap_frac=0.3, t_cols=2048, ps_cols=2048,
             debug_out=None, cache_salt=0.0, t_bufs=8):
    """One core's SPMD program."""
    nc = bass.Bass()
    x = nc.declare_dram_parameter("x", [N, CI, H, W], F32, isOutput=False)
    w = nc.declare_dram_parameter("w", [CO, CI, 3, 3], F32, isOutput=False)
    gamma = nc.declare_dram_parameter("gamma", [CO], F32, isOutput=False)
    beta = nc.declare_dram_parameter("beta", [CO], F32, isOutput=False)
    selcor_in = nc.declare_dram_parameter("selcor", [128, 128], F32,
                                          isOutput=False)
    out = nc.declare_dram_parameter("out", [N, CO, H, W], F32, isOutput=True)

    t_dt = {"bf16": BF16, "f32r": F32R, "f32": F32}[reduce_dtype]
    n_halves = 8192 // t_cols          # spatial (within-group) split
    jph = 8 // n_halves                # images-per-group per half
    nb = ps_cols // 512                # matmul blocks per psum tile

    with tile.TileContext(nc) as tc, ExitStack() as ctx:
        singles = ctx.enter_context(tc.tile_pool(name="singles", bufs=1))
        tpool = ctx.enter_context(tc.tile_pool(name="tpool", bufs=t_bufs))
        cpool = ctx.enter_context(tc.tile_pool(name="cpool", bufs=3))
        pspool = ctx.enter_context(tc.tile_pool(name="psum", bufs=2, space="PSUM"))
        spool = ctx.enter_context(tc.tile_pool(name="stage2", bufs=1))
        dpool = ctx.enter_context(tc.tile_pool(name="dram", bufs=1, space="DRAM"))

        # ---- constants (dep-free DVE setup first) ----
        sel32 = singles.tile([128, 2], F32)         # ones-reduce weights (fp32)
        nc.vector.memset(sel32[:, :], 0.0)
        nc.vector.memset(sel32[0:64, 0:1], 1.0)
        nc.vector.memset(sel32[64:128, 1:2], 1.0)
        if t_dt == F32:
            sel = sel32
        else:
            sel = singles.tile([128, 2], t_dt)      # rounded variant for PE
            nc.vector.tensor_copy(out=sel[:, :], in_=sel32[:, :])
        eps_t = singles.tile([128, 1], F32)
        nc.vector.memset(eps_t[:, :], BN_EPS)
        if cache_salt:
            salt_t = singles.tile([8, 1], F32)
            nc.vector.memset(salt_t[:, :], cache_salt)

        w_sb = singles.tile([128, CO * 9], F32)     # w_sb[(g,ci), co*9+tap]
        w_src = w.rearrange("co ci kh kw -> ci co (kh kw)")
        nc.sync.dma_start(
            out=w_sb[0:64, :].rearrange("p (co t) -> p co t", t=9), in_=w_src)
        nc.sync.dma_start(
            out=w_sb[64:128, :].rearrange("p (co t) -> p co t", t=9), in_=w_src)

        # ---- x padded into SBUF, one tile per half: [(g,ci), j, 34, 34] ----
        auxpool = ctx.enter_context(tc.tile_pool(name="auxpool", bufs=2))
        x_pads = []
        for half in range(n_halves):
            j0 = half * jph
            xp_h = singles.tile([128, jph, PADH, PADW], F32, name=f"xpad_{half}")
            nc.vector.memset(xp_h[:, :, 0, :], 0.0)
            nc.vector.memset(xp_h[:, :, PADH - 1, :], 0.0)
            nc.vector.memset(xp_h[:, :, :, 0], 0.0)
            nc.vector.memset(xp_h[:, :, :, PADW - 1], 0.0)
            for g in range(2):
                for jj in range(jph):
                    nc.sync.dma_start(
                        out=xp_h[g * 64:(g + 1) * 64, jj, 1:H + 1, 1:W + 1],
                        in_=x[g * 8 + j0 + jj])
            x_pads.append(xp_h)

        neg_w_sb = singles.tile([128, CO * 9], F32)
        nc.vector.tensor_scalar(
            out=neg_w_sb[:, :], in0=w_sb[:, :], scalar1=-1.0, scalar2=None,
            op0=mybir.AluOpType.mult)

        # conv scratch in DRAM: [co, g, half, j_local, hw] holds -2*sum(relu)-S_w
        conv_d = dpool.tile([CO, 2, n_halves, jph, HW], F32)
        box_d = dpool.tile([2, n_halves, jph, HW], F32)

        # -S_w[co] bias for evacuation: swb [2, CO]
        wsum_sb = singles.tile([128, CO], F32)
        nc.vector.tensor_reduce(
            out=wsum_sb[:, :],
            in_=w_sb.rearrange("p (co t) -> p co t", t=9),
            axis=mybir.AxisListType.X, op=mybir.AluOpType.add)
        ps_sw = pspool.tile([2, CO], F32, tag="ps")
        nc.tensor.matmul(ps_sw[:, :], lhsT=sel32[:, :], rhs=wsum_sb[:, :],
                         start=True, stop=True)  # exact fp32, tiny
        swb = singles.tile([2, CO], F32)
        nc.scalar.mul(swb[:, :], ps_sw[:, :], -1.0)

        # stage-2 reload targets, loaded incrementally during stage 1
        cs_rl = spool.tile([128, HW], F32)      # [(co,n), hw]
        box_rl = spool.tile([128, HW], F32)     # BoxX broadcast per co

        # ---- stage 1: conv ----
        act_every = int(1.0 / act_tap_frac) if act_tap_frac > 0 else 0
        pass_idx = 0
        def emit_box(half, j0, x_aux):
            ps_box = pspool.tile([2, ps_cols], F32, tag="ps", name=f"psbox_{half}")
            for tap in range(9):
                kh, kw = divmod(tap, 3)
                for b in range(nb):
                    a, hb = divmod(b, 2)   # image-in-half, 16-row block
                    rhs = x_aux[:, a, kh + hb * 16:kh + hb * 16 + 16,
                                kw:kw + W]
                    nc.tensor.matmul(
                        ps_box[:, b * 512:(b + 1) * 512],
                        lhsT=sel[:, :], rhs=rhs,
                        start=(tap == 0), stop=(tap == 8))
            box_cs = cpool.tile([2, ps_cols], F32, tag="cs", name=f"boxcs_{half}")
            nc.scalar.copy(box_cs[:, :], ps_box[:, :])
            nc.sync.dma_start(
                out=box_d[:, half, :, :],
                in_=box_cs.rearrange("p (a hw) -> p a hw", hw=HW))

        for half in range(n_halves):
            j0 = half * jph
            x_pad = x_pads[half]
            if t_dt == F32:
                x_aux = x_pad
            else:
                x_aux = auxpool.tile([128, jph, PADH, PADW], t_dt, tag="aux",
                                     name=f"aux_{half}")
                nc.vector.tensor_copy(
                    out=x_aux.rearrange("p a h w -> p (a h w)"),
                    in_=x_pad.rearrange("p a h w -> p (a h w)"))

            for co in range(CO):
                if co == 4:
                    emit_box(half, j0, x_aux)
                ps = pspool.tile([2, ps_cols], F32, tag="ps", name=f"ps_{co}_{half}")
                for tap in range(9):
                    kh, kw = divmod(tap, 3)
                    k = co * 9 + tap
                    t = tpool.tile([128, jph, H, W], t_dt, tag="t",
                                   name=f"t_{co}_{half}_{tap}")
                    src = x_pad[:, :, kh:kh + H, kw:kw + W]
                    use_act = act_every and (pass_idx % act_every == act_every - 1)
                    pass_idx += 1
                    if use_act:
                        nc.scalar.activation(
                            out=t[:, :, :, :], in_=src,
                            func=mybir.ActivationFunctionType.Relu,
                            bias=neg_w_sb[:, k:k + 1], scale=1.0)
                    else:
                        nc.vector.tensor_scalar(
                            out=t[:, :, :, :], in0=src,
                            scalar1=w_sb[:, k:k + 1], scalar2=0.0,
                            op0=mybir.AluOpType.subtract,
                            op1=mybir.AluOpType.max)
                    tf = t.rearrange("p a h w -> p (a h w)")
                    for b in range(nb):
                        nc.tensor.matmul(
                            ps[:, b * 512:(b + 1) * 512],
                            lhsT=sel[:, :],
                            rhs=tf[:, b * 512:(b + 1) * 512],
                            start=(tap == 0), stop=(tap == 8))
                cs = cpool.tile([2, ps_cols], F32, tag="cs", name=f"cs_{co}_{half}")
                nc.scalar.activation(
                    out=cs[:, :], in_=ps[:, :],
                    func=mybir.ActivationFunctionType.Identity,
                    bias=swb[:, co:co + 1], scale=-2.0)
                nc.sync.dma_start(
                    out=conv_d[co, :, half, :, :],
                    in_=cs.rearrange("p (a hw) -> p a hw", hw=HW))
                if half == n_halves - 1:
                    nc.sync.dma_start(
                        out=cs_rl[co * 16:(co + 1) * 16, :],
                        in_=conv_d[co].rearrange("g h j w -> (g h j) w"))
                    if co == 5:
                        for c2 in range(CO):
                            nc.sync.dma_start(
                                out=box_rl[c2 * 16:(c2 + 1) * 16, :],
                                in_=box_d.rearrange("g h j w -> (g h j) w"))

        # ---- stage 2: BN stats + affine + relu ----
        selcor = singles.tile([128, 128], F32)      # replicated stats selector
        nc.sync.dma_start(out=selcor[:, :], in_=selcor_in[:, :])
        if t_dt == F32:
            selcor_r = selcor
        else:
            selcor_r = singles.tile([128, 128], t_dt)
            nc.vector.tensor_copy(out=selcor_r[:, :], in_=selcor[:, :])
        gam = singles.tile([128, 1], F32)
        bet = singles.tile([128, 1], F32)
        for co in range(CO):
            nc.sync.dma_start(out=gam[co * 16:(co + 1) * 16, :],
                              in_=gamma[co:co + 1].partition_broadcast(16))
            nc.sync.dma_start(out=bet[co * 16:(co + 1) * 16, :],
                              in_=beta[co:co + 1].partition_broadcast(16))
        conv_rl = spool.tile([128, HW], F32)    # true conv output
        nc.vector.tensor_add(conv_rl[:, :], cs_rl[:, :], box_rl[:, :])

        # stats: mean via replicated-selector matmul, then centered var
        if t_dt == F32:
            conv_r = conv_rl
        else:
            conv_r = spool.tile([128, HW], t_dt)
            nc.vector.tensor_copy(out=conv_r[:, :], in_=conv_rl[:, :])
        ps1 = pspool.tile([128, 512], F32, tag="ps")
        ps1b = pspool.tile([128, 512], F32, tag="ps")
        nc.tensor.matmul(ps1[:, :], lhsT=selcor_r[:, :], rhs=conv_r[:, 0:512],
                         start=True, stop=True)
        nc.tensor.matmul(ps1b[:, :], lhsT=selcor_r[:, :], rhs=conv_r[:, 512:HW],
                         start=True, stop=True)
        s1 = spool.tile([128, 1], F32)
        s1b = spool.tile([128, 1], F32)
        nc.vector.tensor_reduce(out=s1[:, :], in_=ps1[:, :],
                                axis=mybir.AxisListType.X, op=mybir.AluOpType.add)
        nc.vector.tensor_reduce(out=s1b[:, :], in_=ps1b[:, :],
                                axis=mybir.AxisListType.X, op=mybir.AluOpType.add)
        inv_n = 1.0 / (N * HW)
        mean = spool.tile([128, 1], F32)
        nc.vector.tensor_scalar(out=mean[:, :], in0=s1[:, :], scalar1=s1b[:, :],
                                scalar2=inv_n, op0=mybir.AluOpType.add,
                                op1=mybir.AluOpType.mult)
        # centered square -> variance without cancellation
        dctr = spool.tile([128, HW], F32)
        nc.vector.tensor_scalar(out=dctr[:, :], in0=conv_rl[:, :],
                                scalar1=mean[:, :], scalar2=None,
                                op0=mybir.AluOpType.subtract)
        sq = spool.tile([128, HW], t_dt)
        nc.scalar.activation(out=sq[:, :], in_=dctr[:, :],
                             func=mybir.ActivationFunctionType.Square)
        ps2 = pspool.tile([128, 512], F32, tag="ps")
        ps2b = pspool.tile([128, 512], F32, tag="ps")
        nc.tensor.matmul(ps2[:, :], lhsT=selcor_r[:, :], rhs=sq[:, 0:512],
                         start=True, stop=True)
        nc.tensor.matmul(ps2b[:, :], lhsT=selcor_r[:, :], rhs=sq[:, 512:HW],
                         start=True, stop=True)
        s2 = spool.tile([128, 1], F32)
        s2b = spool.tile([128, 1], F32)
        nc.vector.tensor_reduce(out=s2[:, :], in_=ps2[:, :],
                                axis=mybir.AxisListType.X, op=mybir.AluOpType.add)
        nc.vector.tensor_reduce(out=s2b[:, :], in_=ps2b[:, :],
                                axis=mybir.AxisListType.X, op=mybir.AluOpType.add)
        var = spool.tile([128, 1], F32)
        nc.vector.tensor_scalar(out=var[:, :], in0=s2[:, :], scalar1=s2b[:, :],
                                scalar2=inv_n, op0=mybir.AluOpType.add,
                                op1=mybir.AluOpType.mult)
        std = spool.tile([128, 1], F32)
        nc.scalar.activation(out=std[:, :], in_=var[:, :],
                             func=mybir.ActivationFunctionType.Sqrt,
                             bias=eps_t[:, :], scale=1.0)
        rstd = spool.tile([128, 1], F32)
        nc.vector.reciprocal(out=rstd[:, :], in_=std[:, :])
        a_t = spool.tile([128, 1], F32)
        nc.vector.tensor_mul(a_t[:, :], gam[:, :], rstd[:, :])
        ma = spool.tile([128, 1], F32)
        nc.vector.tensor_mul(ma[:, :], mean[:, :], a_t[:, :])
        b_t = spool.tile([128, 1], F32)
        nc.vector.tensor_sub(b_t[:, :], bet[:, :], ma[:, :])

        outt = spool.tile([128, HW], F32)
        if debug_out == "conv":
            nc.vector.tensor_copy(out=outt[:, :], in_=conv_rl[:, :])
        elif debug_out == "cs":
            nc.vector.tensor_copy(out=outt[:, :], in_=cs_rl[:, :])
        elif debug_out == "box":
            nc.vector.tensor_copy(out=outt[:, :], in_=box_rl[:, :])
        else:
            nc.scalar.activation(out=outt[:, :], in_=conv_rl[:, :],
                                 func=mybir.ActivationFunctionType.Relu,
                                 bias=b_t[:, :], scale=a_t[:, :])
        out_r = out.rearrange("n co h w -> co n (h w)")
        for co in range(CO):
            nc.sync.dma_start(out=out_r[co], in_=outt[co * 16:(co + 1) * 16, :])

    split_multiwaits(nc)
    return nc


def make_in_maps(x, weight, gamma, beta):
    x = np.ascontiguousarray(x, dtype=np.float32)
    weight = np.ascontiguousarray(weight, dtype=np.float32)
    gamma = np.ascontiguousarray(gamma, dtype=np.float32)
    beta = np.ascontiguousarray(beta, dtype=np.float32)
    selcor = np.zeros((128, 128), np.float32)
    for c in range(CO):
        selcor[c * 16:(c + 1) * 16, c * 16:(c + 1) * 16] = 1.0
    maps = []
    for c in range(8):
        sl = slice(c * CO, (c + 1) * CO)
        maps.append({
            "x": x,
            "w": np.ascontiguousarray(weight[sl]),
            "gamma": np.ascontiguousarray(gamma[sl]),
            "beta": np.ascontiguousarray(beta[sl]),
            "selcor": selcor,
        })
    return maps


def assemble(results):
    return np.concatenate([r["out"] for r in results], axis=1)


# ---------------------------------------------------------------------------
# Harness entry point: full inputs in, full output out.
# Sharding: output channels co split 8 ways (8 channels per NeuronCore);
# BN statistics are over the full batch, which each core owns for its
# channels, so no collectives are needed.
# ---------------------------------------------------------------------------
from concourse.bass_utils import run_bass_kernel_spmd

_NC_CACHE = None


def _get_nc():
    global _NC_CACHE
    if _NC_CACHE is None:
        _NC_CACHE = build_nc()
    return _NC_CACHE


def kernel(x, weight, gamma, beta):
    nc = _get_nc()
    in_maps = make_in_maps(np.asarray(x), np.asarray(weight),
                           np.asarray(gamma), np.asarray(beta))
    res = run_bass_kernel_spmd(nc, in_maps, core_ids=list(range(8)))
    return assemble(res.results)



# revision 2
# speedup vs baseline: 1.5285x; 1.5285x over previous
"""AdderNet BasicBlock kernel for Trainium2, co-sharded across 8 cores.

Per core (co-shard CO=8 of 64 output channels):
  conv[co,n,p] = -sum_{ci,kh,kw} |x[n,ci,p+k-1] - w[co,ci,kh,kw]|   (pad=1)
  BN train-mode over (n,h,w) per co, then ReLU.

Formulation: |d| = 2*relu(d) - d with d = x - w, so
  conv = -2*sum(relu(x-w)) + BoxX - S_w
    BoxX = sum_{ci,tap} x_patch  (PE ones-matmuls on x directly)
    S_w  = sum_{ci,tap} w[co]    (precomputed on host, applied in stage 2)

v2 design:
  - bf16 elementwise: DVE tensor_scalar 4x mode (2 shifted copies of padded
    x keep every tap view 4B-aligned); ACT takes a tuned fraction.
  - 4-way column-tiled concurrent PE reduce: each round of 4 streams maps
    to PE column strips 0/32/64/96 (tile_position), output rows 32j:32j+2
    of a shared [128,2048] PSUM tile; 4 rhs streams flow concurrently.
  - sel weights are -2 for conv streams (+1 for box), so PSUM holds the
    -2*sum(relu) term directly and evacuation is a plain copy.
"""
from contextlib import ExitStack

import numpy as np

import concourse.bass as bass
import concourse.tile as tile
import concourse.mybir as mybir

F32 = mybir.dt.float32
BF16 = mybir.dt.bfloat16
F32R = mybir.dt.float32r
BN_EPS = 1e-5

N, CI, H, W = 16, 64, 32, 32
CO = 8          # output channels per core
HW = H * W      # 1024
PADH, PADW = H + 2, W + 2  # 34
JPH = 2         # images per group per half
NH = 4          # halves (JPH*NH*2groups = 16 images)
TCOLS = JPH * HW            # 2048 free cols per tap tile
NB = TCOLS // 512           # 512-col psum blocks


def split_multiwaits(nc, max_waits=1):
    """This container's walrus rejects >1 semaphore wait per instruction.
    Hoist extras into standalone NoOps on the same (in-order) engine."""
    n_split = 0
    for f in nc.m.functions:
        for b in f.blocks:
            insts = list(b.instructions)
            changed = False
            new = []
            for inst in insts:
                si = inst.sync_info
                waits = list(si.on_wait) if si and si.on_wait else []
                if len(waits) > max_waits:
                    changed = True
                    n_split += 1
                    for w in waits[: len(waits) - max_waits]:
                        new.append(mybir.InstNoOp(
                            name=nc.get_next_instruction_name(),
                            engine=inst.engine, ins=[], outs=[],
                            sync_info=mybir.SyncInfo(on_wait=[w], on_update=[]),
                        ))
                    inst.sync_info = mybir.SyncInfo(
                        on_wait=waits[len(waits) - max_waits:],
                        on_update=list(si.on_update) if si.on_update else [],
                    )
                new.append(inst)
            if changed:
                b.instructions = new
    return n_split


def build_nc(act_frac=0.22, debug_out=None):
    """One core's SPMD program."""
    nc = bass.Bass()
    x = nc.declare_dram_parameter("x", [N, CI, H, W], F32, isOutput=False)
    w = nc.declare_dram_parameter("w", [CO, CI, 3, 3], F32, isOutput=False)
    wneg = nc.declare_dram_parameter("wneg", [CO, CI, 3, 3], F32,
                                     isOutput=False)
    swneg = nc.declare_dram_parameter("swneg", [CO], F32, isOutput=False)
    gamma = nc.declare_dram_parameter("gamma", [CO], F32, isOutput=False)
    beta = nc.declare_dram_parameter("beta", [CO], F32, isOutput=False)
    selcor_in = nc.declare_dram_parameter("selcor", [128, 128], F32,
                                          isOutput=False)
    out = nc.declare_dram_parameter("out", [N, CO, H, W], F32, isOutput=True)

    # stream list: 9 per half (8 conv channels + 1 box), 9 rounds of 4
    streams = []
    for h in range(NH):
        for co in range(CO):
            streams.append(("co", co, h))
        streams.append(("box", None, h))
    n_rounds = len(streams) // 4  # 9

    with tile.TileContext(nc) as tc, ExitStack() as ctx:
        singles = ctx.enter_context(tc.tile_pool(name="singles", bufs=1))
        tpool = ctx.enter_context(tc.tile_pool(name="tpool", bufs=8))
        cpool = ctx.enter_context(tc.tile_pool(name="cpool", bufs=3))
        pspool = ctx.enter_context(tc.tile_pool(name="psum", bufs=2,
                                                space="PSUM"))
        spool = ctx.enter_context(tc.tile_pool(name="stage2", bufs=1))
        dpool = ctx.enter_context(tc.tile_pool(name="dram", bufs=1,
                                               space="DRAM"))

        # ---- PE selector weights (bf16, exact) ----
        selm2 = singles.tile([128, 2], BF16)     # -2 * group-reduce
        nc.vector.memset(selm2[:, :], 0.0)
        nc.vector.memset(selm2[0:64, 0:1], -2.0)
        nc.vector.memset(selm2[64:128, 1:2], -2.0)
        selp1 = singles.tile([128, 2], BF16)     # +1 * group-reduce (box)
        nc.vector.memset(selp1[:, :], 0.0)
        nc.vector.memset(selp1[0:64, 0:1], 1.0)
        nc.vector.memset(selp1[64:128, 1:2], 1.0)
        eps_t = singles.tile([128, 1], F32)
        nc.vector.memset(eps_t[:, :], BN_EPS)

        # ---- weights: w_sb[(g,ci), co*9+tap], plus negated for ACT bias ----
        w_sb = singles.tile([128, CO * 9], F32)
        neg_w_sb = singles.tile([128, CO * 9], F32)
        w_src = w.rearrange("co ci kh kw -> ci co (kh kw)")
        wneg_src = wneg.rearrange("co ci kh kw -> ci co (kh kw)")
        for g in range(2):
            nc.sync.dma_start(
                out=w_sb[g * 64:(g + 1) * 64, :].rearrange(
                    "p (co t) -> p co t", t=9), in_=w_src)
            nc.sync.dma_start(
                out=neg_w_sb[g * 64:(g + 1) * 64, :].rearrange(
                    "p (co t) -> p co t", t=9), in_=wneg_src)

        # swb2[co*16+n] = -S_w[co]  (per-partition bias for stage 2)
        swb2 = singles.tile([128, 1], F32)
        gam = singles.tile([128, 1], F32)
        bet = singles.tile([128, 1], F32)
        for co in range(CO):
            nc.sync.dma_start(out=swb2[co * 16:(co + 1) * 16, :],
                              in_=swneg[co:co + 1].partition_broadcast(16))
            nc.sync.dma_start(out=gam[co * 16:(co + 1) * 16, :],
                              in_=gamma[co:co + 1].partition_broadcast(16))
            nc.sync.dma_start(out=bet[co * 16:(co + 1) * 16, :],
                              in_=beta[co:co + 1].partition_broadcast(16))

        # ---- x: fp32 landing pads + two bf16 shifted copies ----
        # aux0[p, a, h, w]   = xpad (kw=0,2 taps: 4B-aligned rows)
        # aux1[p, a, h, 0:32] = xpad[.., 1:33]  (kw=1 taps)
        x_pads, aux0s, aux1s = [], [], []
        for half in range(NH):
            j0 = half * JPH
            xp = singles.tile([128, JPH, PADH, PADW], F32, name=f"xp_{half}")
            nc.vector.memset(xp[:, :, 0, :], 0.0)
            nc.vector.memset(xp[:, :, PADH - 1, :], 0.0)
            nc.vector.memset(xp[:, :, :, 0], 0.0)
            nc.vector.memset(xp[:, :, :, PADW - 1], 0.0)
            for g in range(2):
                for jj in range(JPH):
                    nc.sync.dma_start(
                        out=xp[g * 64:(g + 1) * 64, jj, 1:H + 1, 1:W + 1],
                        in_=x[g * 8 + j0 + jj])
            a0 = singles.tile([128, JPH, PADH, PADW], BF16, name=f"a0_{half}")
            nc.vector.tensor_copy(
                out=a0.rearrange("p a h w -> p (a h w)"),
                in_=xp.rearrange("p a h w -> p (a h w)"))
            a1 = singles.tile([128, JPH, PADH, W], BF16, name=f"a1_{half}")
            nc.vector.tensor_copy(
                out=a1.rearrange("p a h w -> p (a h w)"),
                in_=a0[:, :, :, 1:1 + W])
            x_pads.append(xp)
            aux0s.append(a0)
            aux1s.append(a1)

        def tap_src(half, kh, kw):
            """bf16 view of the (kh,kw)-shifted window, 4B-aligned."""
            if kw == 1:
                return aux1s[half][:, :, kh:kh + H, 0:W]
            return aux0s[half][:, :, kh:kh + H, kw:kw + W]

        def box_rhs(half, kh, kw, b):
            a, hb = divmod(b, 2)
            if kw == 1:
                return aux1s[half][:, a, kh + hb * 16:kh + hb * 16 + 16, 0:W]
            return aux0s[half][:, a, kh + hb * 16:kh + hb * 16 + 16,
                               kw:kw + W]

        # conv scratch in DRAM
        conv_d = dpool.tile([CO, 2, NH, JPH, HW], F32)
        box_d = dpool.tile([2, NH, JPH, HW], F32)

        # stage-2 reload targets, loaded incrementally
        cs_rl = spool.tile([128, HW], F32)      # [(co,n), hw] = -2*sum(relu)
        box_rl = spool.tile([128, HW], F32)     # BoxX broadcast per co

        # ---- stage 1: rounds of 4 concurrent streams ----
        acc = 0.0
        co_last_round = {}
        for s, (kind, co, h) in enumerate(streams):
            if kind == "co":
                co_last_round[co] = s // 4
        box_done_round = (len(streams) - 1) // 4

        for r in range(n_rounds):
            rs = streams[4 * r:4 * r + 4]
            ps = pspool.tile([128, TCOLS], F32, tag="ps", name=f"ps_{r}")
            t_tiles = {}
            for tap in range(9):
                kh, kw = divmod(tap, 3)
                for j, (kind, co, h) in enumerate(rs):
                    if kind != "co":
                        continue
                    t = tpool.tile([128, JPH, H, W], BF16, tag="t",
                                   name=f"t_{r}_{j}_{tap}")
                    src = tap_src(h, kh, kw)
                    k = co * 9 + tap
                    acc += act_frac
                    if acc >= 1.0:
                        acc -= 1.0
                        nc.scalar.activation(
                            out=t[:, :, :, :], in_=src,
                            func=mybir.ActivationFunctionType.Relu,
                            bias=neg_w_sb[:, k:k + 1], scale=1.0)
                    else:
                        nc.vector.tensor_scalar(
                            out=t[:, :, :, :], in0=src,
                            scalar1=w_sb[:, k:k + 1], scalar2=0.0,
                            op0=mybir.AluOpType.subtract,
                            op1=mybir.AluOpType.max)
                    t_tiles[j] = t.rearrange("p a h w -> p (a h w)")
                for b in range(NB):
                    for j, (kind, co, h) in enumerate(rs):
                        if kind == "co":
                            rhs = t_tiles[j][:, b * 512:(b + 1) * 512]
                            sel = selm2
                        else:
                            rhs = box_rhs(h, kh, kw, b)
                            sel = selp1
                        nc.tensor.matmul(
                            ps[32 * j:32 * j + 2, b * 512:(b + 1) * 512],
                            lhsT=sel[:, :], rhs=rhs,
                            start=(tap == 0), stop=(tap == 8),
                            tile_position=(0, 32 * j))

            cs = cpool.tile([128, TCOLS], F32, tag="cs", name=f"cs_{r}")
            nc.scalar.copy(cs[:, :], ps[:, :])
            for j, (kind, co, h) in enumerate(rs):
                strip = cs[32 * j:32 * j + 2, :].rearrange(
                    "p (a hw) -> p a hw", hw=HW)
                if kind == "co":
                    nc.sync.dma_start(out=conv_d[co, :, h, :, :], in_=strip)
                else:
                    nc.sync.dma_start(out=box_d[:, h, :, :], in_=strip)
            # incremental stage-2 reloads as soon as a channel completes
            for co2, lr in co_last_round.items():
                if lr == r:
                    nc.sync.dma_start(
                        out=cs_rl[co2 * 16:(co2 + 1) * 16, :],
                        in_=conv_d[co2].rearrange("g h j w -> (g h j) w"))
            if r == box_done_round:
                for c2 in range(CO):
                    nc.sync.dma_start(
                        out=box_rl[c2 * 16:(c2 + 1) * 16, :],
                        in_=box_d.rearrange("g h j w -> (g h j) w"))

        # ---- stage 2: BN stats + affine + relu ----
        selcor = singles.tile([128, 128], F32)      # replicated stats selector
        nc.sync.dma_start(out=selcor[:, :], in_=selcor_in[:, :])
        selcor_r = singles.tile([128, 128], F32R)
        nc.vector.tensor_copy(out=selcor_r[:, :], in_=selcor[:, :])

        # conv = cs_rl + box_rl - S_w
        tmp = spool.tile([128, HW], F32)
        nc.vector.tensor_scalar(out=tmp[:, :], in0=cs_rl[:, :],
                                scalar1=swb2[:, :], scalar2=None,
                                op0=mybir.AluOpType.add)
        conv_rl = spool.tile([128, HW], F32)
        nc.vector.tensor_add(conv_rl[:, :], tmp[:, :], box_rl[:, :])

        conv_r = spool.tile([128, HW], F32R)
        nc.vector.tensor_copy(out=conv_r[:, :], in_=conv_rl[:, :])
        ps1 = pspool.tile([128, 512], F32, tag="ps")
        ps1b = pspool.tile([128, 512], F32, tag="ps")
        nc.tensor.matmul(ps1[:, :], lhsT=selcor_r[:, :], rhs=conv_r[:, 0:512],
                         start=True, stop=True)
        nc.tensor.matmul(ps1b[:, :], lhsT=selcor_r[:, :],
                         rhs=conv_r[:, 512:HW], start=True, stop=True)
        s1 = spool.tile([128, 1], F32)
        s1b = spool.tile([128, 1], F32)
        nc.vector.tensor_reduce(out=s1[:, :], in_=ps1[:, :],
                                axis=mybir.AxisListType.X,
                                op=mybir.AluOpType.add)
        nc.vector.tensor_reduce(out=s1b[:, :], in_=ps1b[:, :],
                                axis=mybir.AxisListType.X,
                                op=mybir.AluOpType.add)
        inv_n = 1.0 / (N * HW)
        mean = spool.tile([128, 1], F32)
        nc.vector.tensor_scalar(out=mean[:, :], in0=s1[:, :],
                                scalar1=s1b[:, :], scalar2=inv_n,
                                op0=mybir.AluOpType.add,
                                op1=mybir.AluOpType.mult)
        # centered square -> variance without cancellation
        dctr = spool.tile([128, HW], F32)
        nc.vector.tensor_scalar(out=dctr[:, :], in0=conv_rl[:, :],
                                scalar1=mean[:, :], scalar2=None,
                                op0=mybir.AluOpType.subtract)
        sq = spool.tile([128, HW], F32R)
        nc.scalar.activation(out=sq[:, :], in_=dctr[:, :],
                             func=mybir.ActivationFunctionType.Square)
        ps2 = pspool.tile([128, 512], F32, tag="ps")
        ps2b = pspool.tile([128, 512], F32, tag="ps")
        nc.tensor.matmul(ps2[:, :], lhsT=selcor_r[:, :], rhs=sq[:, 0:512],
                         start=True, stop=True)
        nc.tensor.matmul(ps2b[:, :], lhsT=selcor_r[:, :], rhs=sq[:, 512:HW],
                         start=True, stop=True)
        s2 = spool.tile([128, 1], F32)
        s2b = spool.tile([128, 1], F32)
        nc.vector.tensor_reduce(out=s2[:, :], in_=ps2[:, :],
                                axis=mybir.AxisListType.X,
                                op=mybir.AluOpType.add)
        nc.vector.tensor_reduce(out=s2b[:, :], in_=ps2b[:, :],
                                axis=mybir.AxisListType.X,
                                op=mybir.AluOpType.add)
        var = spool.tile([128, 1], F32)
        nc.vector.tensor_scalar(out=var[:, :], in0=s2[:, :],
                                scalar1=s2b[:, :], scalar2=inv_n,
                                op0=mybir.AluOpType.add,
                                op1=mybir.AluOpType.mult)
        std = spool.tile([128, 1], F32)
        nc.scalar.activation(out=std[:, :], in_=var[:, :],
                             func=mybir.ActivationFunctionType.Sqrt,
                             bias=eps_t[:, :], scale=1.0)
        rstd = spool.tile([128, 1], F32)
        nc.vector.reciprocal(out=rstd[:, :], in_=std[:, :])
        a_t = spool.tile([128, 1], F32)
        nc.vector.tensor_mul(a_t[:, :], gam[:, :], rstd[:, :])
        ma = spool.tile([128, 1], F32)
        nc.vector.tensor_mul(ma[:, :], mean[:, :], a_t[:, :])
        b_t = spool.tile([128, 1], F32)
        nc.vector.tensor_sub(b_t[:, :], bet[:, :], ma[:, :])

        outt = spool.tile([128, HW], F32)
        if debug_out == "conv":
            nc.vector.tensor_copy(out=outt[:, :], in_=conv_rl[:, :])
        elif debug_out == "cs":
            nc.vector.tensor_copy(out=outt[:, :], in_=cs_rl[:, :])
        elif debug_out == "box":
            nc.vector.tensor_copy(out=outt[:, :], in_=box_rl[:, :])
        else:
            nc.scalar.activation(out=outt[:, :], in_=conv_rl[:, :],
                                 func=mybir.ActivationFunctionType.Relu,
                                 bias=b_t[:, :], scale=a_t[:, :])
        out_r = out.rearrange("n co h w -> co n (h w)")
        for co in range(CO):
            nc.sync.dma_start(out=out_r[co], in_=outt[co * 16:(co + 1) * 16, :])

    split_multiwaits(nc)
    return nc


def make_in_maps(x, weight, gamma, beta):
    x = np.ascontiguousarray(x, dtype=np.float32)
    weight = np.ascontiguousarray(weight, dtype=np.float32)
    gamma = np.ascontiguousarray(gamma, dtype=np.float32)
    beta = np.ascontiguousarray(beta, dtype=np.float32)
    selcor = np.zeros((128, 128), np.float32)
    for c in range(CO):
        selcor[c * 16:(c + 1) * 16, c * 16:(c + 1) * 16] = 1.0
    maps = []
    for c in range(8):
        sl = slice(c * CO, (c + 1) * CO)
        wsl = np.ascontiguousarray(weight[sl])
        maps.append({
            "x": x,
            "w": wsl,
            "wneg": np.ascontiguousarray(-wsl),
            "swneg": np.ascontiguousarray(-wsl.sum(axis=(1, 2, 3))),
            "gamma": np.ascontiguousarray(gamma[sl]),
            "beta": np.ascontiguousarray(beta[sl]),
            "selcor": selcor,
        })
    return maps


def assemble(results):
    return np.concatenate([r["out"] for r in results], axis=1)


# ---------------------------------------------------------------------------
# Harness entry point: full inputs in, full output out.
# Sharding: output channels co split 8 ways (8 channels per NeuronCore);
# BN statistics are over the full batch, which each core owns for its
# channels, so no collectives are needed.
# ---------------------------------------------------------------------------
from concourse.bass_utils import run_bass_kernel_spmd

_NC_CACHE = None


def _get_nc():
    global _NC_CACHE
    if _NC_CACHE is None:
        _NC_CACHE = build_nc()
    return _NC_CACHE


def kernel(x, weight, gamma, beta):
    nc = _get_nc()
    in_maps = make_in_maps(np.asarray(x), np.asarray(weight),
                           np.asarray(gamma), np.asarray(beta))
    res = run_bass_kernel_spmd(nc, in_maps, core_ids=list(range(8)))
    return assemble(res.results)


# revision 14
# speedup vs baseline: 1.6190x; 1.0593x over previous
"""AdderNet BasicBlock kernel for Trainium2, co-sharded across 8 cores.

Per core (co-shard CO=8 of 64 output channels):
  conv[co,n,p] = -sum_{ci,kh,kw} |x[n,ci,p+k-1] - w[co,ci,kh,kw]|   (pad=1)
  BN train-mode over (n,h,w) per co, then ReLU.

Formulation: |d| = 2*relu(d) - d with d = x - w, so
  conv = -2*sum(relu(x-w)) + BoxX - S_w
    BoxX = sum_{ci,tap} x_patch  (PE ones-matmuls on x directly)
    S_w  = sum_{ci,tap} w[co]    (precomputed on host, applied in stage 2)

Design:
  - x is pre-padded / pre-transposed / pre-bf16 on the host, in two copies
    (xb shifted by one element) so every tap view is 4B-aligned and the DVE
    tensor_scalar relu runs in 4x mode; ACT takes a tuned fraction.
  - 4-way column-tiled concurrent PE reduce: rounds of 4 streams map to PE
    column strips 0/32/64/96 (tile_position); each stream accumulates its 9
    taps into rows 32j:32j+2 of a shared [128,4096] PSUM tile.
  - sel weights are -2 for conv streams (+1 for box), so PSUM holds the
    -2*sum(relu) term directly; evacuation is one ACT copy per round and
    SBUF->SBUF DMAs redistribute straight into the stage-2 layout
    [(co,n), hw] (no DRAM bounce).
  - stage 2 uses accum_out / tensor_tensor_reduce so BN stats need only
    two tiny N=1 matmuls for the cross-partition sums.
"""
from contextlib import ExitStack

import numpy as np

import concourse.bass as bass
import concourse.tile as tile
import concourse.mybir as mybir

F32 = mybir.dt.float32
BF16 = mybir.dt.bfloat16
F32R = mybir.dt.float32r
BN_EPS = 1e-5

N, CI, H, W = 16, 64, 32, 32
CO = 8          # output channels per core
HW = H * W      # 1024
PADH, PADW = H + 2, W + 2  # 34
JPH = 4         # images per group per half
NH = 2          # halves (JPH*NH*2groups = 16 images)
TCOLS = JPH * HW            # 4096 free cols per tap tile
NB = TCOLS // 512           # 512-col psum blocks


def split_multiwaits(nc, max_waits=1):
    """This container's walrus rejects >1 semaphore wait per instruction.
    Hoist extras into standalone NoOps on the same (in-order) engine."""
    n_split = 0
    for f in nc.m.functions:
        for b in f.blocks:
            insts = list(b.instructions)
            changed = False
            new = []
            for inst in insts:
                si = inst.sync_info
                waits = list(si.on_wait) if si and si.on_wait else []
                if len(waits) > max_waits:
                    changed = True
                    n_split += 1
                    for w in waits[: len(waits) - max_waits]:
                        new.append(mybir.InstNoOp(
                            name=nc.get_next_instruction_name(),
                            engine=inst.engine, ins=[], outs=[],
                            sync_info=mybir.SyncInfo(on_wait=[w], on_update=[]),
                        ))
                    inst.sync_info = mybir.SyncInfo(
                        on_wait=waits[len(waits) - max_waits:],
                        on_update=list(si.on_update) if si.on_update else [],
                    )
                new.append(inst)
            if changed:
                b.instructions = new
    return n_split


def build_nc(act_frac=0.23, debug_out=None):
    """One core's SPMD program."""
    nc = bass.Bass()
    xa = nc.declare_dram_parameter("xa", [CI, N, PADH, PADW], BF16,
                                   isOutput=False)
    xb = nc.declare_dram_parameter("xb", [CI, N, PADH, W], BF16,
                                   isOutput=False)
    w = nc.declare_dram_parameter("w", [CO, CI, 3, 3], F32, isOutput=False)
    wneg = nc.declare_dram_parameter("wneg", [CO, CI, 3, 3], F32,
                                     isOutput=False)
    swneg = nc.declare_dram_parameter("swneg", [CO], F32, isOutput=False)
    gamma = nc.declare_dram_parameter("gamma", [CO], F32, isOutput=False)
    beta = nc.declare_dram_parameter("beta", [CO], F32, isOutput=False)
    selcor_in = nc.declare_dram_parameter("selcor", [128, 128], F32,
                                          isOutput=False)
    out = nc.declare_dram_parameter("out", [N, CO, H, W], F32, isOutput=True)

    # stream list: 9 per half (8 conv channels + 1 box); rounds of 4
    streams = []
    for h in range(NH):
        for co in range(CO):
            streams.append(("co", co, h))
        streams.append(("box", None, h))
    n_rounds = (len(streams) + 3) // 4  # 5 (last round has 2 streams)

    with tile.TileContext(nc) as tc, ExitStack() as ctx:
        singles = ctx.enter_context(tc.tile_pool(name="singles", bufs=1))
        tpool = ctx.enter_context(tc.tile_pool(name="tpool", bufs=8))
        cpool = ctx.enter_context(tc.tile_pool(name="cpool", bufs=2))
        pspool = ctx.enter_context(tc.tile_pool(name="psum", bufs=1,
                                                space="PSUM"))
        spool = ctx.enter_context(tc.tile_pool(name="stage2", bufs=1))

        # ---- weights first (needed by the first tap tiles) ----
        w_sb = singles.tile([128, CO * 9], F32)
        neg_w_sb = singles.tile([128, CO * 9], F32)
        w_src = w.rearrange("co ci kh kw -> ci co (kh kw)")
        wneg_src = wneg.rearrange("co ci kh kw -> ci co (kh kw)")
        for g in range(2):
            nc.sync.dma_start(
                out=w_sb[g * 64:(g + 1) * 64, :].rearrange(
                    "p (co t) -> p co t", t=9), in_=w_src)
            nc.sync.dma_start(
                out=neg_w_sb[g * 64:(g + 1) * 64, :].rearrange(
                    "p (co t) -> p co t", t=9), in_=wneg_src)

        # ---- PE selector weights (bf16, exact) ----
        selm2 = singles.tile([128, 2], BF16)     # -2 * group-reduce
        nc.vector.memset(selm2[:, :], 0.0)
        nc.vector.memset(selm2[0:64, 0:1], -2.0)
        nc.vector.memset(selm2[64:128, 1:2], -2.0)
        selp1 = singles.tile([128, 2], BF16)     # +1 * group-reduce (box)
        nc.vector.memset(selp1[:, :], 0.0)
        nc.vector.memset(selp1[0:64, 0:1], 1.0)
        nc.vector.memset(selp1[64:128, 1:2], 1.0)
        eps_t = singles.tile([128, 1], F32)
        nc.vector.memset(eps_t[:, :], BN_EPS)

        # ---- x: two pre-padded bf16 copies, per half ----
        aux0s, aux1s = [], []
        for half in range(NH):
            j0 = half * JPH
            a0 = singles.tile([128, JPH, PADH, PADW], BF16, name=f"a0_{half}")
            a1 = singles.tile([128, JPH, PADH, W], BF16, name=f"a1_{half}")
            for g in range(2):
                nc.sync.dma_start(out=a0[g * 64:(g + 1) * 64],
                                  in_=xa[:, g * 8 + j0:g * 8 + j0 + JPH])
                nc.sync.dma_start(out=a1[g * 64:(g + 1) * 64],
                                  in_=xb[:, g * 8 + j0:g * 8 + j0 + JPH])
            aux0s.append(a0)
            aux1s.append(a1)

        # swb2[co*16+n] = -S_w[co]  (per-partition bias for stage 2)
        swb2 = singles.tile([128, 1], F32)
        gam = singles.tile([128, 1], F32)
        bet = singles.tile([128, 1], F32)
        for co in range(CO):
            nc.sync.dma_start(out=swb2[co * 16:(co + 1) * 16, :],
                              in_=swneg[co:co + 1].partition_broadcast(16))
            nc.sync.dma_start(out=gam[co * 16:(co + 1) * 16, :],
                              in_=gamma[co:co + 1].partition_broadcast(16))
            nc.sync.dma_start(out=bet[co * 16:(co + 1) * 16, :],
                              in_=beta[co:co + 1].partition_broadcast(16))
        selcor = singles.tile([128, 128], F32)      # replicated stats selector
        nc.sync.dma_start(out=selcor[:, :], in_=selcor_in[:, :])

        def tap_src(half, kh, kw):
            """bf16 view of the (kh,kw)-shifted window, 4B-aligned."""
            if kw == 1:
                return aux1s[half][:, :, kh:kh + H, 0:W]
            return aux0s[half][:, :, kh:kh + H, kw:kw + W]

        def box_rhs(half, kh, kw, b):
            a, hb = divmod(b, 2)
            if kw == 1:
                return aux1s[half][:, a, kh + hb * 16:kh + hb * 16 + 16, 0:W]
            return aux0s[half][:, a, kh + hb * 16:kh + hb * 16 + 16,
                               kw:kw + W]

        # conv scratch in DRAM (partition-crossing redistribution)
        dpool = ctx.enter_context(tc.tile_pool(name="dram", bufs=1,
                                               space="DRAM"))
        conv_d = dpool.tile([CO, 2, NH, JPH, HW], F32)
        box_d = dpool.tile([2, NH, JPH, HW], F32)

        # stage-2 reload targets, loaded incrementally
        cs_rl = spool.tile([128, HW], F32)      # [(co,n), hw] = -2*sum(relu)
        box_rl = spool.tile([128, HW], F32)     # BoxX broadcast per co

        # ---- stage 1: rounds of up to 4 concurrent streams ----
        acc = 0.0
        co_last_round = {}
        box_last_round = 0
        for s, (kind, co, h) in enumerate(streams):
            if kind == "co":
                co_last_round[co] = s // 4
            else:
                box_last_round = s // 4
        for r in range(n_rounds):
            rs = streams[4 * r:4 * r + 4]
            ps = pspool.tile([128, TCOLS], F32, tag="ps", name=f"ps_{r}")
            t_tiles = {}
            for tap in range(9):
                kh, kw = divmod(tap, 3)
                for j, (kind, co, h) in enumerate(rs):
                    if kind != "co":
                        continue
                    t = tpool.tile([128, JPH, H, W], BF16, tag="t",
                                   name=f"t_{r}_{j}_{tap}")
                    src = tap_src(h, kh, kw)
                    k = co * 9 + tap
                    acc += act_frac
                    if acc >= 1.0:
                        acc -= 1.0
                        nc.scalar.activation(
                            out=t[:, :, :, :], in_=src,
                            func=mybir.ActivationFunctionType.Relu,
                            bias=neg_w_sb[:, k:k + 1], scale=1.0)
                    else:
                        nc.vector.tensor_scalar(
                            out=t[:, :, :, :], in0=src,
                            scalar1=w_sb[:, k:k + 1], scalar2=0.0,
                            op0=mybir.AluOpType.subtract,
                            op1=mybir.AluOpType.max)
                    t_tiles[j] = t.rearrange("p a h w -> p (a h w)")
                for b in range(NB):
                    for j, (kind, co, h) in enumerate(rs):
                        if kind == "co":
                            rhs = t_tiles[j][:, b * 512:(b + 1) * 512]
                            sel = selm2
                        else:
                            rhs = box_rhs(h, kh, kw, b)
                            sel = selp1
                        nc.tensor.matmul(
                            ps[32 * j:32 * j + 2, b * 512:(b + 1) * 512],
                            lhsT=sel[:, :], rhs=rhs,
                            start=(tap == 0), stop=(tap == 8),
                            tile_position=(0, 32 * j))

            cs = cpool.tile([128, TCOLS], F32, tag="cs", name=f"cs_{r}")
            nc.scalar.copy(cs[:, :], ps[:, :])
            for j, (kind, co, h) in enumerate(rs):
                strip = cs[32 * j:32 * j + 2, :].rearrange(
                    "p (a hw) -> p a hw", hw=HW)
                if kind == "co":
                    nc.sync.dma_start(out=conv_d[co, :, h, :, :], in_=strip)
                else:
                    nc.sync.dma_start(out=box_d[:, h, :, :], in_=strip)
            # incremental stage-2 reloads as soon as a channel completes
            for co2, lr in co_last_round.items():
                if lr == r:
                    nc.sync.dma_start(
                        out=cs_rl[co2 * 16:(co2 + 1) * 16, :],
                        in_=conv_d[co2].rearrange("g h j w -> (g h j) w"))
            if r == box_last_round:
                for c2 in range(CO):
                    nc.sync.dma_start(
                        out=box_rl[c2 * 16:(c2 + 1) * 16, :],
                        in_=box_d.rearrange("g h j w -> (g h j) w"))

        # ---- stage 2: BN stats + affine + relu ----
        # conv = (cs_rl + (-S_w)) + box_rl, with free per-partition row sums
        conv_rl = spool.tile([128, HW], F32)
        psums = spool.tile([128, 1], F32)
        nc.vector.scalar_tensor_tensor(
            out=conv_rl[:, :], in0=cs_rl[:, :], scalar=swb2[:, :],
            in1=box_rl[:, :], op0=mybir.AluOpType.add,
            op1=mybir.AluOpType.add, accum_out=psums[:, :])

        inv_n = 1.0 / (N * HW)
        ps1 = pspool.tile([128, 1], F32, tag="ps")
        nc.tensor.matmul(ps1[:, :], lhsT=selcor[:, :], rhs=psums[:, :],
                         start=True, stop=True)
        mean = spool.tile([128, 1], F32)
        nc.vector.tensor_scalar(out=mean[:, :], in0=ps1[:, :],
                                scalar1=inv_n, scalar2=None,
                                op0=mybir.AluOpType.mult)
        # centered square-sum per partition -> variance without cancellation
        dctr = spool.tile([128, HW], F32)
        nc.vector.tensor_scalar(out=dctr[:, :], in0=conv_rl[:, :],
                                scalar1=mean[:, :], scalar2=None,
                                op0=mybir.AluOpType.subtract)
        sq = spool.tile([128, HW], F32)
        pssq = spool.tile([128, 1], F32)
        nc.vector.scalar_tensor_tensor(
            out=sq[:, :], in0=dctr[:, :], scalar=1.0, in1=dctr[:, :],
            op0=mybir.AluOpType.mult, op1=mybir.AluOpType.mult,
            accum_out=pssq[:, :])
        ps2 = pspool.tile([128, 1], F32, tag="ps")
        nc.tensor.matmul(ps2[:, :], lhsT=selcor[:, :], rhs=pssq[:, :],
                         start=True, stop=True)
        var = spool.tile([128, 1], F32)
        nc.vector.tensor_scalar(out=var[:, :], in0=ps2[:, :],
                                scalar1=inv_n, scalar2=None,
                                op0=mybir.AluOpType.mult)
        std = spool.tile([128, 1], F32)
        nc.scalar.activation(out=std[:, :], in_=var[:, :],
                             func=mybir.ActivationFunctionType.Sqrt,
                             bias=eps_t[:, :], scale=1.0)
        rstd = spool.tile([128, 1], F32)
        nc.vector.reciprocal(out=rstd[:, :], in_=std[:, :])
        a_t = spool.tile([128, 1], F32)
        nc.vector.tensor_mul(a_t[:, :], gam[:, :], rstd[:, :])
        ma = spool.tile([128, 1], F32)
        nc.vector.tensor_mul(ma[:, :], mean[:, :], a_t[:, :])
        b_t = spool.tile([128, 1], F32)
        nc.vector.tensor_sub(b_t[:, :], bet[:, :], ma[:, :])

        outt = spool.tile([128, HW], F32)
        if debug_out == "conv":
            nc.vector.tensor_copy(out=outt[:, :], in_=conv_rl[:, :])
        elif debug_out == "cs":
            nc.vector.tensor_copy(out=outt[:, :], in_=cs_rl[:, :])
        elif debug_out == "box":
            nc.vector.tensor_copy(out=outt[:, :], in_=box_rl[:, :])
        else:
            nc.scalar.activation(out=outt[:, :], in_=conv_rl[:, :],
                                 func=mybir.ActivationFunctionType.Relu,
                                 bias=b_t[:, :], scale=a_t[:, :])
        out_r = out.rearrange("n co h w -> co n (h w)")
        for co in range(CO):
            nc.sync.dma_start(out=out_r[co], in_=outt[co * 16:(co + 1) * 16, :])

    split_multiwaits(nc)
    return nc


def make_in_maps(x, weight, gamma, beta):
    import ml_dtypes
    x = np.ascontiguousarray(x, dtype=np.float32)
    weight = np.ascontiguousarray(weight, dtype=np.float32)
    gamma = np.ascontiguousarray(gamma, dtype=np.float32)
    beta = np.ascontiguousarray(beta, dtype=np.float32)
    # pre-padded, ci-major, bf16 copies of x (xb shifted one element in w)
    xpad = np.zeros((CI, N, PADH, PADW), np.float32)
    xpad[:, :, 1:H + 1, 1:W + 1] = x.transpose(1, 0, 2, 3)
    xa = np.ascontiguousarray(xpad.astype(ml_dtypes.bfloat16))
    xb = np.ascontiguousarray(xa[:, :, :, 1:1 + W])
    selcor = np.zeros((128, 128), np.float32)
    for c in range(CO):
        selcor[c * 16:(c + 1) * 16, c * 16:(c + 1) * 16] = 1.0
    maps = []
    for c in range(8):
        sl = slice(c * CO, (c + 1) * CO)
        wsl = np.ascontiguousarray(weight[sl])
        maps.append({
            "xa": xa,
            "xb": xb,
            "w": wsl,
            "wneg": np.ascontiguousarray(-wsl),
            "swneg": np.ascontiguousarray(-wsl.sum(axis=(1, 2, 3))),
            "gamma": np.ascontiguousarray(gamma[sl]),
            "beta": np.ascontiguousarray(beta[sl]),
            "selcor": selcor,
        })
    return maps


def assemble(results):
    return np.concatenate([r["out"] for r in results], axis=1)


# ---------------------------------------------------------------------------
# Harness entry point: full inputs in, full output out.
# Sharding: output channels co split 8 ways (8 channels per NeuronCore);
# BN statistics are over the full batch, which each core owns for its
# channels, so no collectives are needed.
# ---------------------------------------------------------------------------
from concourse.bass_utils import run_bass_kernel_spmd

_NC_CACHE = None


def _get_nc():
    global _NC_CACHE
    if _NC_CACHE is None:
        _NC_CACHE = build_nc()
    return _NC_CACHE


def kernel(x, weight, gamma, beta):
    nc = _get_nc()
    in_maps = make_in_maps(np.asarray(x), np.asarray(weight),
                           np.asarray(gamma), np.asarray(beta))
    res = run_bass_kernel_spmd(nc, in_maps, core_ids=list(range(8)))
    return assemble(res.results)


# revision 21
# speedup vs baseline: 1.7485x; 1.0800x over previous
"""AdderNet BasicBlock kernel for Trainium2, co-sharded across 8 cores.

Per core (co-shard CO=8 of 64 output channels):
  conv[co,n,p] = -sum_{ci,kh,kw} |x[n,ci,p+k-1] - w[co,ci,kh,kw]|   (pad=1)
  BN train-mode over (n,h,w) per co, then ReLU.

Formulation: |d| = 2*relu(d) - d with d = x - w, so
  conv = -2*sum(relu(x-w)) + BoxX - S_w
    BoxX = sum_{ci,tap} x_patch  (PE ones-matmuls on x directly)
    S_w  = sum_{ci,tap} w[co]    (precomputed on host, applied in stage 2)

Design:
  - x is pre-padded / pre-transposed / pre-bf16 on the host, in two copies
    (xb shifted by one element) so every tap view is 4B-aligned and the DVE
    tensor_scalar relu runs in 4x mode; ACT takes a tuned fraction.
  - 4-way column-tiled concurrent PE reduce: rounds of 4 streams map to PE
    column strips 0/32/64/96 (tile_position); each stream accumulates its 9
    taps into rows 32j:32j+2 of a shared [128,4096] PSUM tile.
  - sel weights are -2 for conv streams (+1 for box), so PSUM holds the
    -2*sum(relu) term directly; evacuation is one ACT copy per round and
    SBUF->SBUF DMAs redistribute straight into the stage-2 layout
    [(co,n), hw] (no DRAM bounce).
  - stage 2 uses accum_out / tensor_tensor_reduce so BN stats need only
    two tiny N=1 matmuls for the cross-partition sums.
"""
from contextlib import ExitStack

import numpy as np

import concourse.bass as bass
import concourse.tile as tile
import concourse.mybir as mybir

F32 = mybir.dt.float32
BF16 = mybir.dt.bfloat16
F32R = mybir.dt.float32r
BN_EPS = 1e-5

N, CI, H, W = 16, 64, 32, 32
CO = 8          # output channels per core
HW = H * W      # 1024
PADH, PADW = H + 2, W + 2  # 34
JPH = 4         # images per group per half
NH = 2          # halves (JPH*NH*2groups = 16 images)
TCOLS = JPH * HW            # 4096 free cols per tap tile
NB = TCOLS // 512           # 512-col psum blocks


def split_multiwaits(nc, max_waits=1):
    """This container's walrus rejects >1 semaphore wait per instruction.
    Hoist extras into standalone NoOps on the same (in-order) engine."""
    n_split = 0
    for f in nc.m.functions:
        for b in f.blocks:
            insts = list(b.instructions)
            changed = False
            new = []
            for inst in insts:
                si = inst.sync_info
                waits = list(si.on_wait) if si and si.on_wait else []
                if len(waits) > max_waits:
                    changed = True
                    n_split += 1
                    for w in waits[: len(waits) - max_waits]:
                        new.append(mybir.InstNoOp(
                            name=nc.get_next_instruction_name(),
                            engine=inst.engine, ins=[], outs=[],
                            sync_info=mybir.SyncInfo(on_wait=[w], on_update=[]),
                        ))
                    inst.sync_info = mybir.SyncInfo(
                        on_wait=waits[len(waits) - max_waits:],
                        on_update=list(si.on_update) if si.on_update else [],
                    )
                new.append(inst)
            if changed:
                b.instructions = new
    return n_split


def build_nc(act_frac=0.23, debug_out=None):
    """One core's SPMD program."""
    nc = bass.Bass()
    xa = nc.declare_dram_parameter("xa", [CI, N, PADH, PADW], BF16,
                                   isOutput=False)
    xb = nc.declare_dram_parameter("xb", [CI, N, PADH, W], BF16,
                                   isOutput=False)
    w = nc.declare_dram_parameter("w", [CO, CI, 3, 3], F32, isOutput=False)
    wneg = nc.declare_dram_parameter("wneg", [CO, CI, 3, 3], F32,
                                     isOutput=False)
    swneg = nc.declare_dram_parameter("swneg", [CO], F32, isOutput=False)
    gamma = nc.declare_dram_parameter("gamma", [CO], F32, isOutput=False)
    beta = nc.declare_dram_parameter("beta", [CO], F32, isOutput=False)
    selcor_in = nc.declare_dram_parameter("selcor", [128, 128], F32,
                                          isOutput=False)
    out = nc.declare_dram_parameter("out", [N, CO, H, W], F32, isOutput=True)

    # stream list: 9 per half (1 box + 8 conv channels); rounds of 4
    streams = []
    for h in range(NH):
        streams.append(("box", None, h))
        for co in range(CO):
            streams.append(("co", co, h))
    n_rounds = (len(streams) + 3) // 4  # 5 (last round has 2 streams)

    with tile.TileContext(nc) as tc, ExitStack() as ctx:
        singles = ctx.enter_context(tc.tile_pool(name="singles", bufs=1))
        tpool = ctx.enter_context(tc.tile_pool(name="tpool", bufs=8))
        cpool = ctx.enter_context(tc.tile_pool(name="cpool", bufs=2))
        pspool = ctx.enter_context(tc.tile_pool(name="psumA", bufs=1,
                                                space="PSUM"))
        pspoolB = ctx.enter_context(tc.tile_pool(name="psumB", bufs=1,
                                                 space="PSUM"))
        spool = ctx.enter_context(tc.tile_pool(name="stage2", bufs=1))

        # ---- weights first (needed by the first tap tiles) ----
        w_sb = singles.tile([128, CO * 9], F32)
        neg_w_sb = singles.tile([128, CO * 9], F32)
        w_src = w.rearrange("co ci kh kw -> ci co (kh kw)")
        wneg_src = wneg.rearrange("co ci kh kw -> ci co (kh kw)")
        for g in range(2):
            nc.sync.dma_start(
                out=w_sb[g * 64:(g + 1) * 64, :].rearrange(
                    "p (co t) -> p co t", t=9), in_=w_src)
            nc.sync.dma_start(
                out=neg_w_sb[g * 64:(g + 1) * 64, :].rearrange(
                    "p (co t) -> p co t", t=9), in_=wneg_src)

        # ---- PE selector weights (bf16, exact): -2 * group-reduce for ALL
        # streams (box too; fixed up with a -0.5 scale in stage 2) so the
        # stationary weights never change.
        selm2 = singles.tile([128, 2], BF16)
        nc.vector.memset(selm2[:, :], 0.0)
        nc.vector.memset(selm2[0:64, 0:1], -2.0)
        nc.vector.memset(selm2[64:128, 1:2], -2.0)
        eps_t = singles.tile([128, 1], F32)
        nc.vector.memset(eps_t[:, :], BN_EPS)

        # ---- x: two pre-padded bf16 copies, per half ----
        aux0s, aux1s = [], []
        for half in range(NH):
            j0 = half * JPH
            a0 = singles.tile([128, JPH, PADH, PADW], BF16, name=f"a0_{half}")
            a1 = singles.tile([128, JPH, PADH, W], BF16, name=f"a1_{half}")
            for g in range(2):
                nc.sync.dma_start(out=a0[g * 64:(g + 1) * 64],
                                  in_=xa[:, g * 8 + j0:g * 8 + j0 + JPH])
                nc.sync.dma_start(out=a1[g * 64:(g + 1) * 64],
                                  in_=xb[:, g * 8 + j0:g * 8 + j0 + JPH])
            aux0s.append(a0)
            aux1s.append(a1)

        gam = singles.tile([128, 1], F32)
        bet = singles.tile([128, 1], F32)
        for co in range(CO):
            nc.sync.dma_start(out=gam[co * 16:(co + 1) * 16, :],
                              in_=gamma[co:co + 1].partition_broadcast(16))
            nc.sync.dma_start(out=bet[co * 16:(co + 1) * 16, :],
                              in_=beta[co:co + 1].partition_broadcast(16))
        selcor = singles.tile([128, 128], F32)      # replicated stats selector
        nc.sync.dma_start(out=selcor[:, :], in_=selcor_in[:, :])

        def tap_src(half, kh, kw):
            """bf16 view of the (kh,kw)-shifted window, 4B-aligned."""
            if kw == 1:
                return aux1s[half][:, :, kh:kh + H, 0:W]
            return aux0s[half][:, :, kh:kh + H, kw:kw + W]

        def box_rhs(half, kh, kw, b):
            a, hb = divmod(b, 2)
            if kw == 1:
                return aux1s[half][:, a, kh + hb * 16:kh + hb * 16 + 16, 0:W]
            return aux0s[half][:, a, kh + hb * 16:kh + hb * 16 + 16,
                               kw:kw + W]

        # conv scratch in DRAM (partition-crossing redistribution)
        dpool = ctx.enter_context(tc.tile_pool(name="dram", bufs=1,
                                               space="DRAM"))
        conv_d = dpool.tile([CO, 2, NH, JPH, HW], F32)
        box_d = dpool.tile([2, NH, JPH, HW], F32)

        # stage-2 reload targets, loaded incrementally
        cs_rl = spool.tile([128, HW], F32)      # [(co,n), hw] = -2*sum(relu)
        box_rl = spool.tile([128, HW], F32)     # BoxX broadcast per co

        # ---- stage 1: rounds of up to 4 concurrent streams ----
        acc = 0.0
        co_last_round = {}
        box_last_round = 0
        for s, (kind, co, h) in enumerate(streams):
            if kind == "co":
                co_last_round[co] = s // 4
            else:
                box_last_round = s // 4
        HB = TCOLS // 2  # psum half-tile cols (2048)
        for r in range(n_rounds):
            rs = streams[4 * r:4 * r + 4]
            # two psum half-tiles (images 0-1 / 2-3): next round can start
            # in half A while half B is still evacuating
            psA = pspool.tile([128, HB], F32, tag="ps", name=f"psA_{r}")
            psB = pspoolB.tile([128, HB], F32, tag="psb", name=f"psB_{r}")

            def emit_mm(tap, b_range):
                kh, kw = divmod(tap, 3)
                for b in b_range:
                    ps = psA if b < NB // 2 else psB
                    col = (b % (NB // 2)) * 512
                    for j, (kind, co, h) in enumerate(rs):
                        if kind == "co":
                            rhs = t_tiles[j][:, b * 512:(b + 1) * 512]
                        else:
                            rhs = box_rhs(h, kh, kw, b)
                        nc.tensor.matmul(
                            ps[32 * j:32 * j + 2, col:col + 512],
                            lhsT=selm2[:, :], rhs=rhs,
                            start=(tap == 0), stop=(tap == 8),
                            tile_position=(0, 32 * j))

            t_tiles = {}
            for tap in range(9):
                kh, kw = divmod(tap, 3)
                for j, (kind, co, h) in enumerate(rs):
                    if kind != "co":
                        continue
                    t = tpool.tile([128, JPH, H, W], BF16, tag="t",
                                   name=f"t_{r}_{j}_{tap}")
                    src = tap_src(h, kh, kw)
                    k = co * 9 + tap
                    acc += act_frac
                    if acc >= 1.0:
                        acc -= 1.0
                        nc.scalar.activation(
                            out=t[:, :, :, :], in_=src,
                            func=mybir.ActivationFunctionType.Relu,
                            bias=neg_w_sb[:, k:k + 1], scale=1.0)
                    else:
                        nc.vector.tensor_scalar(
                            out=t[:, :, :, :], in0=src,
                            scalar1=w_sb[:, k:k + 1], scalar2=0.0,
                            op0=mybir.AluOpType.subtract,
                            op1=mybir.AluOpType.max)
                    t_tiles[j] = t.rearrange("p a h w -> p (a h w)")
                if tap < 8:
                    emit_mm(tap, range(NB))
                else:
                    emit_mm(tap, range(NB // 2))

            csA = cpool.tile([128, HB], F32, tag="cs", name=f"csA_{r}")
            nc.scalar.copy(csA[:, :], psA[:, :])
            emit_mm(8, range(NB // 2, NB))
            csB = cpool.tile([128, HB], F32, tag="cs", name=f"csB_{r}")
            nc.scalar.copy(csB[:, :], psB[:, :])
            for j, (kind, co, h) in enumerate(rs):
                for half_i, cs in enumerate((csA, csB)):
                    strip = cs[32 * j:32 * j + 2, :].rearrange(
                        "p (a hw) -> p a hw", hw=HW)
                    asl = slice(half_i * 2, half_i * 2 + 2)
                    if kind == "co":
                        nc.sync.dma_start(out=conv_d[co, :, h, asl, :],
                                          in_=strip)
                    else:
                        nc.sync.dma_start(out=box_d[:, h, asl, :], in_=strip)
            # incremental stage-2 reloads as soon as a channel completes
            for co2, lr in co_last_round.items():
                if lr == r:
                    nc.sync.dma_start(
                        out=cs_rl[co2 * 16:(co2 + 1) * 16, :],
                        in_=conv_d[co2].rearrange("g h j w -> (g h j) w"))
            if r == box_last_round:
                for c2 in range(CO):
                    nc.sync.dma_start(
                        out=box_rl[c2 * 16:(c2 + 1) * 16, :],
                        in_=box_d.rearrange("g h j w -> (g h j) w"))

        # ---- stage 2: BN stats + affine + relu ----
        # conv0 = cs_rl - 0.5*box_rl = conv + S_w (the S_w shift cancels in
        # BN: using mean0 = mean(conv0) everywhere is equivalent), with free
        # per-partition row sums via accum_out
        conv_rl = spool.tile([128, HW], F32)
        psums = spool.tile([128, 1], F32)
        nc.vector.scalar_tensor_tensor(
            out=conv_rl[:, :], in0=box_rl[:, :], scalar=-0.5,
            in1=cs_rl[:, :], op0=mybir.AluOpType.mult,
            op1=mybir.AluOpType.add, accum_out=psums[:, :])

        inv_n = 1.0 / (N * HW)
        ps1 = pspool.tile([128, 1], F32, tag="ps")
        nc.tensor.matmul(ps1[:, :], lhsT=selcor[:, :], rhs=psums[:, :],
                         start=True, stop=True)
        mean = spool.tile([128, 1], F32)
        nc.vector.tensor_scalar(out=mean[:, :], in0=ps1[:, :],
                                scalar1=inv_n, scalar2=None,
                                op0=mybir.AluOpType.mult)
        # centered square-sum per partition -> variance without cancellation
        dctr = spool.tile([128, HW], F32)
        nc.vector.tensor_scalar(out=dctr[:, :], in0=conv_rl[:, :],
                                scalar1=mean[:, :], scalar2=None,
                                op0=mybir.AluOpType.subtract)
        sq = spool.tile([128, HW], F32)
        pssq = spool.tile([128, 1], F32)
        nc.vector.scalar_tensor_tensor(
            out=sq[:, :], in0=dctr[:, :], scalar=1.0, in1=dctr[:, :],
            op0=mybir.AluOpType.mult, op1=mybir.AluOpType.mult,
            accum_out=pssq[:, :])
        ps2 = pspool.tile([128, 1], F32, tag="ps")
        nc.tensor.matmul(ps2[:, :], lhsT=selcor[:, :], rhs=pssq[:, :],
                         start=True, stop=True)
        var = spool.tile([128, 1], F32)
        nc.vector.tensor_scalar(out=var[:, :], in0=ps2[:, :],
                                scalar1=inv_n, scalar2=None,
                                op0=mybir.AluOpType.mult)
        std = spool.tile([128, 1], F32)
        nc.scalar.activation(out=std[:, :], in_=var[:, :],
                             func=mybir.ActivationFunctionType.Sqrt,
                             bias=eps_t[:, :], scale=1.0)
        rstd = spool.tile([128, 1], F32)
        nc.vector.reciprocal(out=rstd[:, :], in_=std[:, :])
        a_t = spool.tile([128, 1], F32)
        nc.vector.tensor_mul(a_t[:, :], gam[:, :], rstd[:, :])
        ma = spool.tile([128, 1], F32)
        nc.vector.tensor_mul(ma[:, :], mean[:, :], a_t[:, :])
        b_t = spool.tile([128, 1], F32)
        nc.vector.tensor_sub(b_t[:, :], bet[:, :], ma[:, :])

        outt = spool.tile([128, HW], F32)
        if debug_out == "conv":
            nc.vector.tensor_copy(out=outt[:, :], in_=conv_rl[:, :])
        elif debug_out == "cs":
            nc.vector.tensor_copy(out=outt[:, :], in_=cs_rl[:, :])
        elif debug_out == "box":
            nc.vector.tensor_copy(out=outt[:, :], in_=box_rl[:, :])
        else:
            nc.scalar.activation(out=outt[:, :], in_=conv_rl[:, :],
                                 func=mybir.ActivationFunctionType.Relu,
                                 bias=b_t[:, :], scale=a_t[:, :])
        out_r = out.rearrange("n co h w -> co n (h w)")
        for co in range(CO):
            nc.sync.dma_start(out=out_r[co], in_=outt[co * 16:(co + 1) * 16, :])

    split_multiwaits(nc)
    return nc


def make_in_maps(x, weight, gamma, beta):
    import ml_dtypes
    x = np.ascontiguousarray(x, dtype=np.float32)
    weight = np.ascontiguousarray(weight, dtype=np.float32)
    gamma = np.ascontiguousarray(gamma, dtype=np.float32)
    beta = np.ascontiguousarray(beta, dtype=np.float32)
    # pre-padded, ci-major, bf16 copies of x (xb shifted one element in w)
    xpad = np.zeros((CI, N, PADH, PADW), np.float32)
    xpad[:, :, 1:H + 1, 1:W + 1] = x.transpose(1, 0, 2, 3)
    xa = np.ascontiguousarray(xpad.astype(ml_dtypes.bfloat16))
    xb = np.ascontiguousarray(xa[:, :, :, 1:1 + W])
    selcor = np.zeros((128, 128), np.float32)
    for c in range(CO):
        selcor[c * 16:(c + 1) * 16, c * 16:(c + 1) * 16] = 1.0
    maps = []
    for c in range(8):
        sl = slice(c * CO, (c + 1) * CO)
        wsl = np.ascontiguousarray(weight[sl])
        maps.append({
            "xa": xa,
            "xb": xb,
            "w": wsl,
            "wneg": np.ascontiguousarray(-wsl),
            "swneg": np.ascontiguousarray(-wsl.sum(axis=(1, 2, 3))),
            "gamma": np.ascontiguousarray(gamma[sl]),
            "beta": np.ascontiguousarray(beta[sl]),
            "selcor": selcor,
        })
    return maps


def assemble(results):
    return np.concatenate([r["out"] for r in results], axis=1)


# ---------------------------------------------------------------------------
# Harness entry point: full inputs in, full output out.
# Sharding: output channels co split 8 ways (8 channels per NeuronCore);
# BN statistics are over the full batch, which each core owns for its
# channels, so no collectives are needed.
# ---------------------------------------------------------------------------
from concourse.bass_utils import run_bass_kernel_spmd

_NC_CACHE = None


def _get_nc():
    global _NC_CACHE
    if _NC_CACHE is None:
        _NC_CACHE = build_nc()
    return _NC_CACHE


def kernel(x, weight, gamma, beta):
    nc = _get_nc()
    in_maps = make_in_maps(np.asarray(x), np.asarray(weight),
                           np.asarray(gamma), np.asarray(beta))
    res = run_bass_kernel_spmd(nc, in_maps, core_ids=list(range(8)))
    return assemble(res.results)
